# revision 6
# baseline (speedup 1.0000x reference)
"""LIF current-encoder (norse lif_current_encoder, 32 steps) on 8 Trainium2 cores.

Reference recurrence per element (dt*tau_mem_inv = 0.1, v_leak=v_reset=0, v_th=1):
    v' = 0.9*v + 0.1*X ;  z = (v' >= 1) ;  v = v' * (1 - z)

Closed form: until an element's first spike, v_t = X*(1 - 0.9^t), so
    z_t = (X >= c_t),   c_t = 1 / (1 - 0.9^(t+1))
The c_t are decreasing with c_31 = 1.03549...; for any input with
max(X) < c_31 no element ever spikes, the reset never engages, and the
closed form equals the reference recurrence EXACTLY (the declared input
domain is X in [0,1)).  kernel() guards the domain on the host and falls
back to an exact numpy recurrence for out-of-domain inputs.

Sharding: pure data-parallel over the batch dim (8 batches -> 8 cores).

Per-core program (v4 — dual DMA queue, bf16/u8 frame mix):
  - input X pre-cast to bf16 on the host (RNE, bit-identical to a device
    cast); the 384 KB input DMA is hoisted to the very top of the program
  - frames t0..15 computed as bf16 on DVE (tensor_scalar is_ge, 4x perf
    mode, ~470ns/frame), frames t16..31 as uint8 (2x mode, ~870ns/frame):
    the per-core DMA drain rate is ~470 GB/s aggregate (the two DGE queues
    share the 16 SDMA channels), so halving the bytes of the second half
    of the frames balances DVE (~21.3us) against DMA (~20.9us).
    GpSimd/Pool compute was tried and measured ~23us/frame on HW
    (software Q7 loop, ~100x the cost model) — do NOT put work there
  - output DMAs alternate between BOTH hardware DGE queues (SP and
    Activation sequencers, ~460 GB/s each) in groups of 2 frames, so the
    two queues drain in parallel while DVE fills the next frames
  - every DMA carries a completion-sem increment (walrus codegen SIGABRTs
    without one); nothing waits on it — the Block-exit engine drains cover
    the tail transfers (verified bit-exact on dense-spike inputs)
Host casts to the f32 [T,B,C,H,W] output.  Spike values 0/1 are exact in
bf16, and bf16 rounding of X cannot cross any c_t (X < 1 rounds to at
most 1.0 < 1.0355), so the result is bit-exact.
"""

import sys

sys.path.insert(0, "/opt/trn_rl_repo")

import ml_dtypes
import numpy as np

import concourse.bass as bass
import concourse.mybir as mybir
from concourse import bacc
from concourse.bass_utils import run_bass_kernel_spmd

N_CORES = 8
T = 32
CHW = 3 * 256 * 256
P = 128
F = CHW // P  # 1536

_f32 = mybir.dt.float32
_bf16 = mybir.dt.bfloat16
_u8 = mybir.dt.uint8
_op = mybir.AluOpType

_C = [float(np.float32(1.0 / (1.0 - 0.9 ** (t + 1)))) for t in range(T)]
_DOMAIN_MAX = 1.0 / (1.0 - 0.9**T) - 1e-3

N_BF16 = 16
N_U8 = T - N_BF16
GROUP = 2

_nc_cache = None


def _build_nc():
    nc = bacc.Bacc("TRN2", target_bir_lowering=False, debug=False)
    x = nc.dram_tensor("x", [P, F], _bf16, kind="ExternalInput")
    out_b = nc.dram_tensor("out_b", [N_BF16, CHW], _bf16, kind="ExternalOutput")
    out_u = nc.dram_tensor("out_u", [N_U8, CHW], _u8, kind="ExternalOutput")

    with (
        nc.sbuf_tensor([P, F], _bf16) as xb,
        nc.sbuf_tensor([P, N_BF16 * F], _bf16) as zb,
        nc.sbuf_tensor([P, N_U8 * F], _u8) as zu,
        nc.semaphore("in_sem") as in_sem,
        nc.semaphore("zv_sem") as zv_sem,
        nc.semaphore("dma_sem") as dma_sem,
        nc.Block() as block,
    ):
        # input DMA: emitted outside the block, then hoisted to the top of
        # the entry basic block so the SP sequencer issues it immediately
        in_dma = nc.sync.dma_start(out=xb[:, :], in_=x.ap()[:, :])
        in_dma.then_inc(in_sem, 16)

        bgroups = [(i, min(GROUP, N_BF16 - i)) for i in range(0, N_BF16, GROUP)]
        ugroups = [(i, min(GROUP, N_U8 - i)) for i in range(0, N_U8, GROUP)]

        def dma_bgroup(eng, g0, gn):
            eng.wait_ge(zv_sem, g0 + gn)
            eng.dma_start(
                out=out_b.ap()[g0 : g0 + gn].rearrange("t (p f) -> p t f", p=P),
                in_=zb[:, g0 * F : (g0 + gn) * F].rearrange("p (t f) -> p t f", t=gn),
            ).then_inc(dma_sem, 16)

        def dma_ugroup(eng, k0, kn):
            eng.wait_ge(zv_sem, N_BF16 + k0 + kn)
            eng.dma_start(
                out=out_u.ap()[k0 : k0 + kn].rearrange("t (p f) -> p t f", p=P),
                in_=zu[:, k0 * F : (k0 + kn) * F].rearrange("p (t f) -> p t f", t=kn),
            ).then_inc(dma_sem, 16)

        # alternate frame-group DMAs across the two hardware DGE queues
        @block.sync
        def _(sync):
            for gi, (g0, gn) in enumerate(bgroups):
                if gi % 2 == 0:
                    dma_bgroup(sync, g0, gn)
            for gi, (k0, kn) in enumerate(ugroups):
                if gi % 2 == 0:
                    dma_ugroup(sync, k0, kn)

        @block.scalar
        def _(scalar):
            for gi, (g0, gn) in enumerate(bgroups):
                if gi % 2 == 1:
                    dma_bgroup(scalar, g0, gn)
            for gi, (k0, kn) in enumerate(ugroups):
                if gi % 2 == 1:
                    dma_ugroup(scalar, k0, kn)

        @block.vector
        def _(vector):
            vector.wait_ge(in_sem, 16)
            for t in range(N_BF16):
                nc.vector.tensor_scalar(
                    out=zb[:, t * F : (t + 1) * F],
                    in0=xb[:],
                    scalar1=_C[t],
                    scalar2=None,
                    op0=_op.is_ge,
                ).then_inc(zv_sem, 1)
            for k in range(N_U8):
                nc.vector.tensor_scalar(
                    out=zu[:, k * F : (k + 1) * F],
                    in0=xb[:],
                    scalar1=_C[N_BF16 + k],
                    scalar2=None,
                    op0=_op.is_ge,
                ).then_inc(zv_sem, 1)

    entry = nc.m.functions[0].blocks[0]
    entry.instructions.remove(in_dma.ins)
    entry.instructions.insert(1, in_dma.ins)

    nc.compile()
    return nc


def _get_nc():
    global _nc_cache
    if _nc_cache is None:
        _nc_cache = _build_nc()
    return _nc_cache


def _trace_in_maps(X):
    Xb = np.ascontiguousarray(X, dtype=np.float32).reshape(N_CORES, P, F)
    Xb = Xb.astype(ml_dtypes.bfloat16)
    return [{"x": Xb[b]} for b in range(N_CORES)]


def _numpy_fallback(X: np.ndarray) -> np.ndarray:
    # exact f32 recurrence; only used for inputs outside [0, 1.0345)
    v = np.zeros_like(X)
    zs = np.empty((T,) + X.shape, dtype=np.float32)
    for t in range(T):
        v = v + np.float32(0.1) * ((np.float32(0.0) - v) + X)
        z = (v - np.float32(1.0) >= 0).astype(np.float32)
        zs[t] = z
        v = v - z * v
    return zs


def kernel(X: np.ndarray) -> np.ndarray:
    X = np.ascontiguousarray(X, dtype=np.float32)
    assert X.shape == (N_CORES, 3, 256, 256), X.shape
    if float(X.max()) >= _DOMAIN_MAX:
        return _numpy_fallback(X)
    nc = _get_nc()
    in_maps = _trace_in_maps(X)
    res = run_bass_kernel_spmd(nc, in_maps, list(range(N_CORES)))
    out = np.empty((T, N_CORES, CHW), dtype=np.float32)
    for b in range(N_CORES):
        out[:N_BF16, b] = np.asarray(res.results[b]["out_b"]).astype(np.float32)
        out[N_BF16:, b] = np.asarray(res.results[b]["out_u"]).astype(np.float32)
    return out.reshape(T, N_CORES, 3, 256, 256)


# revision 7
# speedup vs baseline: 1.2309x; 1.2309x over previous
"""LIF current-encoder (norse lif_current_encoder, 32 steps) on 8 Trainium2 cores.

Reference recurrence per element (dt*tau_mem_inv = 0.1, v_leak=v_reset=0, v_th=1):
    v' = 0.9*v + 0.1*X ;  z = (v' >= 1) ;  v = v' * (1 - z)

Closed form: until an element's first spike, v_t = X*(1 - 0.9^t), so
    z_t = (X >= c_t),   c_t = 1 / (1 - 0.9^(t+1))
The c_t are decreasing with c_31 = 1.03549...; for any input with
max(X) < c_31 no element ever spikes, the reset never engages, and the
closed form equals the reference recurrence EXACTLY (the declared input
domain is X in [0,1)).  kernel() guards the domain on the host and falls
back to an exact numpy recurrence for out-of-domain inputs.

Sharding: pure data-parallel over the batch dim (8 batches -> 8 cores).

Per-core program (v3 — dual hardware DMA queue):
  - input X pre-cast to bf16 on the host (RNE, bit-identical to a device
    cast); the 384 KB input DMA is hoisted to the very top of the program
  - all 32 frames computed as bf16 on DVE (tensor_scalar is_ge, 4x perf
    mode, ~460ns/frame, ~14.8us total); GpSimd/Pool compute was tried and
    measured ~23us/frame on HW (software Q7 loop, ~100x the cost model) —
    do NOT put elementwise work there
  - output DMAs alternate between BOTH hardware DGE queues (SP and
    Activation sequencers, ~460 GB/s each) in groups of 2 frames, so the
    two queues drain in parallel while DVE fills the next frames
  - every DMA carries a completion-sem increment (walrus codegen SIGABRTs
    without one); nothing waits on it — the Block-exit engine drains cover
    the tail transfers (verified bit-exact on dense-spike inputs)
Host casts to the f32 [T,B,C,H,W] output.  Spike values 0/1 are exact in
bf16, and bf16 rounding of X cannot cross any c_t (X < 1 rounds to at
most 1.0 < 1.0355), so the result is bit-exact.
"""

import sys

sys.path.insert(0, "/opt/trn_rl_repo")

import ml_dtypes
import numpy as np

import concourse.bass as bass
import concourse.mybir as mybir
from concourse import bacc
from concourse.bass_utils import run_bass_kernel_spmd

N_CORES = 8
T = 32
CHW = 3 * 256 * 256
P = 128
F = CHW // P  # 1536

_f32 = mybir.dt.float32
_bf16 = mybir.dt.bfloat16
_op = mybir.AluOpType

_C = [float(np.float32(1.0 / (1.0 - 0.9 ** (t + 1)))) for t in range(T)]
_DOMAIN_MAX = 1.0 / (1.0 - 0.9**T) - 1e-3

GROUP = 2

_nc_cache = None


def _build_nc():
    nc = bacc.Bacc("TRN2", target_bir_lowering=False, debug=False)
    x = nc.dram_tensor("x", [P, F], _bf16, kind="ExternalInput")
    out_b = nc.dram_tensor("out_b", [T, CHW], _bf16, kind="ExternalOutput")

    with (
        nc.sbuf_tensor([P, F], _bf16) as xb,
        nc.sbuf_tensor([P, T * F], _bf16) as zb,
        nc.semaphore("in_sem") as in_sem,
        nc.semaphore("zv_sem") as zv_sem,
        nc.semaphore("dma_sem") as dma_sem,
        nc.Block() as block,
    ):
        # input DMA: emitted outside the block, then hoisted to the top of
        # the entry basic block so the SP sequencer issues it immediately
        in_dma = nc.sync.dma_start(out=xb[:, :], in_=x.ap()[:, :])
        in_dma.then_inc(in_sem, 16)

        groups = [(i, min(GROUP, T - i)) for i in range(0, T, GROUP)]

        def dma_group(eng, g0, gn):
            eng.wait_ge(zv_sem, g0 + gn)
            eng.dma_start(
                out=out_b.ap()[g0 : g0 + gn].rearrange("t (p f) -> p t f", p=P),
                in_=zb[:, g0 * F : (g0 + gn) * F].rearrange("p (t f) -> p t f", t=gn),
            ).then_inc(dma_sem, 16)

        # alternate frame-group DMAs across the two hardware DGE queues
        @block.sync
        def _(sync):
            for gi, (g0, gn) in enumerate(groups):
                if gi % 2 == 0:
                    dma_group(sync, g0, gn)

        @block.scalar
        def _(scalar):
            for gi, (g0, gn) in enumerate(groups):
                if gi % 2 == 1:
                    dma_group(scalar, g0, gn)

        @block.vector
        def _(vector):
            vector.wait_ge(in_sem, 16)
            for t in range(T):
                nc.vector.tensor_scalar(
                    out=zb[:, t * F : (t + 1) * F],
                    in0=xb[:],
                    scalar1=_C[t],
                    scalar2=None,
                    op0=_op.is_ge,
                ).then_inc(zv_sem, 1)

    entry = nc.m.functions[0].blocks[0]
    entry.instructions.remove(in_dma.ins)
    entry.instructions.insert(1, in_dma.ins)

    nc.compile()
    return nc


def _get_nc():
    global _nc_cache
    if _nc_cache is None:
        _nc_cache = _build_nc()
    return _nc_cache


def _trace_in_maps(X):
    Xb = np.ascontiguousarray(X, dtype=np.float32).reshape(N_CORES, P, F)
    Xb = Xb.astype(ml_dtypes.bfloat16)
    return [{"x": Xb[b]} for b in range(N_CORES)]


def _numpy_fallback(X: np.ndarray) -> np.ndarray:
    # exact f32 recurrence; only used for inputs outside [0, 1.0345)
    v = np.zeros_like(X)
    zs = np.empty((T,) + X.shape, dtype=np.float32)
    for t in range(T):
        v = v + np.float32(0.1) * ((np.float32(0.0) - v) + X)
        z = (v - np.float32(1.0) >= 0).astype(np.float32)
        zs[t] = z
        v = v - z * v
    return zs


def kernel(X: np.ndarray) -> np.ndarray:
    X = np.ascontiguousarray(X, dtype=np.float32)
    assert X.shape == (N_CORES, 3, 256, 256), X.shape
    if float(X.max()) >= _DOMAIN_MAX:
        return _numpy_fallback(X)
    nc = _get_nc()
    in_maps = _trace_in_maps(X)
    res = run_bass_kernel_spmd(nc, in_maps, list(range(N_CORES)))
    out = np.empty((T, N_CORES, CHW), dtype=np.float32)
    for b in range(N_CORES):
        out[:, b] = np.asarray(res.results[b]["out_b"]).astype(np.float32)
    return out.reshape(T, N_CORES, 3, 256, 256)
